# revision 14
# baseline (speedup 1.0000x reference)
"""EnhancedGovernanceAttention on 8 trn2 NeuronCores.

Sharding: tensor-parallel over heads with batch as secondary axis.
Core c handles batch b = c//4 and heads h in [4*(c%4), 4*(c%4)+4).
Each core computes a partial output [S, D] (its heads' contribution to
the out-projection); the host sums the partials and adds the bias.

Device kernel per core:
  - q,k projections computed transposed (qT[d, tok]) so QK^T needs no
    transpose; RoPE applied via a partition-shift SBUF DMA + DVE muls
    (cos/sin tables are host-provided, with the q tables pre-scaled by
    1/sqrt(dh) and the sin tables pre-signed for the rotate-half).
  - v computed in natural layout [tok, d].
  - scores per (head, 128-row tile of queries): one bf16 matmul per
    512-wide key tile (upper-triangle tiles skipped), bias added in
    natural layout: ACT computes log(1 + mem*gs + 1e-8), DVE folds
    prophetic*gs/2 + policy*gs + scores, gpsimd applies the causal
    fill on diagonal tiles via affine_select.
  - exp on ACT (bf16 out) with accum_out accumulating the softmax
    denominator for free.
  - P~ (bf16) transposed 128x128-blockwise via DMA XBAR transposes;
    AV matmul computes attn^T[d, tok] directly (lhsT = v tiles).
  - denominators transposed via a tiny PE transpose; reciprocal on DVE;
    normalization fused into the AV psum->sbuf copy.
  - out-projection over the core's 512 head-dims -> partial [S, D] f32.
"""

import sys

sys.path.insert(0, "/opt/trn_rl_repo")

import math

import ml_dtypes
import numpy as np

# problem shapes (hardcoded per contract)
B, S, D = 2, 1024, 2048
H, DH = 16, 128
GS = 0.1
ROPE_BASE = 10000.0
NCORES = 8
HPC = 4  # heads per core
DLOC = HPC * DH  # 512
TOK = S  # tokens per core (one batch each)
P = 128
KT = D // P  # 16 k-tiles over hidden dim
MASK_F32 = True  # stream governance masks as f32 (False: bf16, halves DMA)

_CACHE = {}


def legalize_sync_waits(nc, max_waits=1):
    """This walrus build only encodes 1 sem wait per instruction; move
    overflow waits onto same-engine NoOps placed immediately before."""
    import concourse.mybir as mybir

    n_split = 0
    for f in nc.m.functions:
        for bb in f.blocks:
            il = bb.instructions
            i = 0
            while i < len(il):
                inst = il[i]
                si = inst.sync_info
                if si is not None and len(si.on_wait) > max_waits:
                    waits = list(si.on_wait)
                    keep = waits[:max_waits]
                    extra = waits[max_waits:]
                    pos = i
                    j = 0
                    while extra:
                        chunk, extra = extra[:max_waits], extra[max_waits:]
                        nop = mybir.InstNoOp(
                            name=f"{inst.name}-swx{j}",
                            engine=inst.engine,
                            bass_nofuse=True,
                            sync_info=mybir.SyncInfo(on_wait=chunk, on_update=[]),
                        )
                        il.insert(pos, nop)
                        pos += 1
                        j += 1
                    inst.sync_info = mybir.SyncInfo(
                        on_wait=keep, on_update=list(si.on_update)
                    )
                    i = pos + 1
                    n_split += 1
                else:
                    i += 1
    return n_split


def build_nc():
    import concourse.bass as bass
    import concourse.mybir as mybir
    from concourse.masks import make_identity
    from concourse.tile import TileContext

    f32 = mybir.dt.float32
    bf16 = mybir.dt.bfloat16
    mask_dt = f32 if MASK_F32 else bf16
    Alu = mybir.AluOpType
    Act = mybir.ActivationFunctionType

    nc = bass.Bass()

    xt = nc.dram_tensor("xt", [D, TOK], bf16, kind="ExternalInput")
    wq = nc.dram_tensor("wq", [D, DLOC], bf16, kind="ExternalInput")
    wk = nc.dram_tensor("wk", [D, DLOC], bf16, kind="ExternalInput")
    wv = nc.dram_tensor("wv", [D, DLOC], bf16, kind="ExternalInput")
    wo = nc.dram_tensor("wo", [DLOC, D], bf16, kind="ExternalInput")
    pm = nc.dram_tensor("pm", [HPC, S, S], mask_dt, kind="ExternalInput")
    pol = nc.dram_tensor("pol", [HPC, S, S], mask_dt, kind="ExternalInput")
    mem = nc.dram_tensor("mem", [HPC, S, S], mask_dt, kind="ExternalInput")
    cosq = nc.dram_tensor("cosq", [P, TOK], f32, kind="ExternalInput")
    sinq = nc.dram_tensor("sinq", [P, TOK], f32, kind="ExternalInput")
    cosk = nc.dram_tensor("cosk", [P, TOK], f32, kind="ExternalInput")
    sink = nc.dram_tensor("sink", [P, TOK], f32, kind="ExternalInput")
    out = nc.dram_tensor("out", [TOK, D], f32, kind="ExternalOutput")

    xt_t = xt.rearrange("(kt p) t -> p kt t", p=P)
    wq_t = wq.rearrange("(kt p) c -> p kt c", p=P)
    wk_t = wk.rearrange("(kt p) c -> p kt c", p=P)
    wv_t = wv.rearrange("(kt p) c -> p kt c", p=P)
    wo_t = wo.rearrange("(kt p) c -> p kt c", p=P)

    with TileContext(nc) as tc:
        with (
            tc.tile_pool(name="persist", bufs=1) as persist,
            tc.tile_pool(name="ppsum", bufs=3, space="PSUM") as ppsum,
            tc.tile_pool(name="qk_psum", bufs=3, space="PSUM") as qk_psum,
            tc.tile_pool(name="av_psum", bufs=2, space="PSUM") as av_psum,
            tc.tile_pool(name="work", bufs=2) as work,
            tc.tile_pool(name="masks", bufs=2) as maskp,
            tc.tile_pool(name="pnatp", bufs=3) as pnatp,
            tc.tile_pool(name="ptp", bufs=1) as ptp,
            tc.tile_pool(name="lp", bufs=2) as lp,
        ):
            # ---- persistent tiles
            qT = persist.tile([P, HPC, TOK], bf16)  # [d, h, tok]
            kT = persist.tile([P, HPC, TOK], bf16)
            v_sb = persist.tile([P, TOK // P, DLOC], bf16)  # [tokp, tokt, hd]
            attnT = persist.tile([P, HPC, TOK], bf16)  # [d, h(=ktile), tok]
            lnbias = persist.tile([P, 1], f32)
            nc.vector.memset(lnbias[:], 1.0 + 1e-8)

            # ---- phase 1: projections (scoped pools free afterwards)
            with tc.tile_pool(name="proj", bufs=1) as proj, tc.tile_pool(
                name="wstream", bufs=2
            ) as wstream:
                xt_sb = proj.tile([P, KT, TOK], bf16)
                nc.sync.dma_start(xt_sb[:], xt_t)
                tabs = {}
                for name, tab in (
                    ("cosq", cosq),
                    ("sinq", sinq),
                    ("cosk", cosk),
                    ("sink", sink),
                ):
                    t = proj.tile([P, TOK], bf16, tag=name)
                    nc.gpsimd.dma_start(t[:], tab[:])  # casts f32 -> bf16
                    tabs[name] = t

                # v natural (resident weights)
                wv_sb = proj.tile([P, KT, DLOC], bf16, tag="wv")
                nc.sync.dma_start(wv_sb[:], wv_t)
                for mt in range(TOK // P):
                    ps = ppsum.tile([P, DLOC], f32, tag="pp")
                    for k in range(KT):
                        nc.tensor.matmul(
                            ps[:],
                            xt_sb[:, k, mt * P : (mt + 1) * P],
                            wv_sb[:, k, :],
                            start=(k == 0),
                            stop=(k == KT - 1),
                        )
                    nc.scalar.copy(v_sb[:, mt, :], ps[:])

                # q/k transposed + rope; weights streamed per head
                for which, wt in (("q", wq_t), ("k", wk_t)):
                    dest = qT if which == "q" else kT
                    cos_t = tabs["cosq" if which == "q" else "cosk"]
                    sin_t = tabs["sinq" if which == "q" else "sink"]
                    for m in range(HPC):  # head
                        w_cur = wstream.tile([P, KT, P], bf16, tag="wqk")
                        nc.sync.dma_start(w_cur[:], wt[:, :, m * P : (m + 1) * P])
                        for n in range(TOK // 512):
                            tsl = slice(n * 512, (n + 1) * 512)
                            ps = ppsum.tile([P, 512], f32, tag="pp")
                            for k in range(KT):
                                nc.tensor.matmul(
                                    ps[:],
                                    w_cur[:, k, :],
                                    xt_sb[:, k, tsl],
                                    start=(k == 0),
                                    stop=(k == KT - 1),
                                )
                            raw = work.tile([P, 512], f32, tag="rope_raw")
                            nc.scalar.copy(raw[:], ps[:])
                            swp = work.tile([P, 512], f32, tag="rope_swp")
                            nc.sync.dma_start(swp[0:64, :], raw[64:128, :])
                            nc.sync.dma_start(swp[64:128, :], raw[0:64, :])
                            t1 = work.tile([P, 512], f32, tag="rope_t1")
                            nc.vector.tensor_tensor(
                                t1[:], ps[:], cos_t[:, tsl], Alu.mult
                            )
                            t2 = work.tile([P, 512], f32, tag="rope_t2")
                            nc.vector.tensor_tensor(
                                t2[:], swp[:], sin_t[:, tsl], Alu.mult
                            )
                            nc.vector.tensor_tensor(
                                dest[:, m, tsl], t1[:], t2[:], Alu.add
                            )

            # ---- phase 2: attention per local head
            NEG = -1.0e30
            for h in range(HPC):
                pT = ptp.tile([P, TOK // P, TOK], bf16, tag="pt")
                l_all = lp.tile([P, TOK // P], f32, tag="l_all")
                for mi in range(TOK // P):
                    n_sk = (mi // 4 + 1) * 512
                    pnat = pnatp.tile([P, TOK], bf16, tag="pnat")
                    for ni in range(n_sk // 512):
                        ksl = slice(ni * 512, (ni + 1) * 512)
                        ps = qk_psum.tile([P, 512], f32, tag="qk")
                        nc.tensor.matmul(
                            ps[:],
                            qT[:, h, mi * P : (mi + 1) * P],
                            kT[:, h, ksl],
                            start=True,
                            stop=True,
                        )
                        mem_t = maskp.tile([P, 512], mask_dt, tag="memt")
                        nc.sync.dma_start(
                            mem_t[:], mem[h, mi * P : (mi + 1) * P, ksl]
                        )
                        pm_t = maskp.tile([P, 512], mask_dt, tag="pmt")
                        nc.sync.dma_start(pm_t[:], pm[h, mi * P : (mi + 1) * P, ksl])
                        pol_t = maskp.tile([P, 512], mask_dt, tag="polt")
                        nc.sync.dma_start(
                            pol_t[:], pol[h, mi * P : (mi + 1) * P, ksl]
                        )
                        logm = work.tile([P, 512], f32, tag="logm")
                        nc.scalar.activation(
                            logm[:], mem_t[:], Act.Ln, bias=lnbias[:], scale=GS
                        )
                        s1 = work.tile([P, 512], f32, tag="s1")
                        nc.vector.scalar_tensor_tensor(
                            s1[:], pm_t[:], GS * 0.5, logm[:], Alu.mult, Alu.add
                        )
                        s2 = work.tile([P, 512], f32, tag="s2")
                        nc.vector.scalar_tensor_tensor(
                            s2[:], pol_t[:], GS, s1[:], Alu.mult, Alu.add
                        )
                        s3 = work.tile([P, 512], f32, tag="s3")
                        nc.vector.tensor_tensor(s3[:], s2[:], ps[:], Alu.add)
                        if ni == mi // 4:  # diagonal tile: causal fill
                            nc.gpsimd.affine_select(
                                out=s3[:],
                                in_=s3[:],
                                compare_op=Alu.is_ge,
                                fill=NEG,
                                base=mi * P - ni * 512,
                                pattern=[[-1, 512]],
                                channel_multiplier=1,
                            )
                        if ni == 0:
                            acc = l_all[:, mi : mi + 1]
                        else:
                            ltmp = lp.tile([P, 1], f32, tag="ltmp")
                            acc = ltmp[:]
                        nc.scalar.activation(
                            pnat[:, ksl],
                            s3[:],
                            Act.Exp,
                            accum_out=acc,
                        )
                        if ni == 1:
                            nc.vector.tensor_add(
                                l_all[:, mi : mi + 1], l_all[:, mi : mi + 1], ltmp[:]
                            )
                    # normalize the row (per-partition 1/l), then transpose
                    # its 128x128 blocks into pT
                    rec = lp.tile([P, 1], f32, tag="rec")
                    nc.vector.reciprocal(rec[:], l_all[:, mi : mi + 1])
                    nc.gpsimd.tensor_scalar_mul(
                        pnat[:, :n_sk], pnat[:, :n_sk], rec[:]
                    )
                    for ki in range(n_sk // P):
                        nc.scalar.dma_start_transpose(
                            pT[:, ki, mi * P : (mi + 1) * P],
                            pnat[:, ki * P : (ki + 1) * P],
                        )
                # AV: attnT[d, tok] accumulating over key tiles
                for nj in range(2):
                    n_ki = 4 if nj == 0 else 8
                    ps = av_psum.tile([P, 512], f32, tag="av")
                    for ki in range(n_ki):
                        nc.tensor.matmul(
                            ps[:],
                            v_sb[:, ki, h * P : (h + 1) * P],
                            pT[:, ki, nj * 512 : (nj + 1) * 512],
                            start=(ki == 0),
                            stop=(ki == n_ki - 1),
                        )
                    nc.scalar.copy(attnT[:, h, nj * 512 : (nj + 1) * 512], ps[:])

            # ---- phase 3: out projection partial (wo streamed per n-chunk)
            with tc.tile_pool(name="outp", bufs=3) as outp, tc.tile_pool(
                name="wop", bufs=2
            ) as wop:
                for n in range(D // 512):
                    wo_sb = wop.tile([P, HPC, 512], bf16, tag="wo")
                    nc.sync.dma_start(
                        wo_sb[:], wo_t[:, :, n * 512 : (n + 1) * 512]
                    )
                    for mt in range(TOK // P):
                        ps = ppsum.tile([P, 512], f32, tag="pp")
                        for kt in range(HPC):
                            nc.tensor.matmul(
                                ps[:],
                                attnT[:, kt, mt * P : (mt + 1) * P],
                                wo_sb[:, kt, :],
                                start=(kt == 0),
                                stop=(kt == HPC - 1),
                            )
                        ot = outp.tile([P, 512], f32, tag="ot")
                        nc.scalar.copy(ot[:], ps[:])
                        nc.sync.dma_start(
                            out[mt * P : (mt + 1) * P, n * 512 : (n + 1) * 512],
                            ot[:],
                        )

    legalize_sync_waits(nc, max_waits=1)
    return nc


def _rope_tables():
    """cos/sin tables in transposed-projection layout [128 dims, TOK],
    with rotate-half sign folded into sin and 1/sqrt(dh) folded into the
    q tables."""
    inv_freq = 1.0 / (
        ROPE_BASE ** (np.arange(0, DH, 2, dtype=np.float32) / DH)
    )  # [64]
    t = np.arange(S, dtype=np.float32)
    freqs = np.outer(t, inv_freq)  # [S, 64]
    cos = np.cos(freqs)
    sin = np.sin(freqs)
    cos2 = np.empty((P, TOK), np.float32)
    sin2 = np.empty((P, TOK), np.float32)
    cos2[0:64] = cos.T
    cos2[64:128] = cos.T
    sin2[0:64] = -sin.T
    sin2[64:128] = sin.T
    scale = 1.0 / math.sqrt(DH)
    return cos2 * scale, sin2 * scale, cos2, sin2


def make_in_maps(x, prophetic_mask, policy_mask, memory_weights, Wq, Wk, Wv, Wo):
    bf16 = ml_dtypes.bfloat16
    mask_np_dt = np.float32 if MASK_F32 else bf16
    cosq, sinq, cosk, sink = _rope_tables()
    wq_b = np.ascontiguousarray(Wq).astype(bf16)
    wk_b = np.ascontiguousarray(Wk).astype(bf16)
    wv_b = np.ascontiguousarray(Wv).astype(bf16)
    wo_b = np.ascontiguousarray(Wo).astype(bf16)
    in_maps = []
    for c in range(NCORES):
        b = c // 4
        g = c % 4
        cols = slice(DLOC * g, DLOC * (g + 1))
        hsl = slice(HPC * g, HPC * (g + 1))
        xt_c = np.ascontiguousarray(x[b].T).astype(bf16)  # [D, TOK]
        in_maps.append(
            {
                "xt": xt_c,
                "wq": np.ascontiguousarray(wq_b[:, cols]),
                "wk": np.ascontiguousarray(wk_b[:, cols]),
                "wv": np.ascontiguousarray(wv_b[:, cols]),
                "wo": np.ascontiguousarray(wo_b[cols, :]),
                "pm": np.ascontiguousarray(prophetic_mask[b, hsl]).astype(
                    mask_np_dt
                ),
                "pol": np.ascontiguousarray(policy_mask[b, hsl]).astype(mask_np_dt),
                "mem": np.ascontiguousarray(memory_weights[b, hsl]).astype(
                    mask_np_dt
                ),
                "cosq": cosq,
                "sinq": sinq,
                "cosk": cosk,
                "sink": sink,
            }
        )
    return in_maps


def kernel(x, prophetic_mask, policy_mask, memory_weights, Wq, Wk, Wv, Wo, bo):
    from concourse.bass_utils import run_bass_kernel_spmd

    if "nc" not in _CACHE:
        _CACHE["nc"] = build_nc()
    nc = _CACHE["nc"]
    in_maps = make_in_maps(
        x, prophetic_mask, policy_mask, memory_weights, Wq, Wk, Wv, Wo
    )
    res = run_bass_kernel_spmd(nc, in_maps, list(range(NCORES)))
    out = np.zeros((B, S, D), np.float32)
    for c in range(NCORES):
        out[c // 4] += res.results[c]["out"]
    out += np.asarray(bo, np.float32)[None, None, :]
    return out


# revision 17
# speedup vs baseline: 9.8892x; 9.8892x over previous
"""EnhancedGovernanceAttention on 8 trn2 NeuronCores.

Sharding: tensor-parallel over heads with batch as secondary axis.
Core c handles batch b = c//4 and heads h in [4*(c%4), 4*(c%4)+4).
Each core computes a partial output [S, D] (its heads' contribution to
the out-projection); the host sums the partials and adds the bias.

Device kernel per core:
  - q,k projections computed transposed (qT[d, tok]) so QK^T needs no
    transpose; RoPE applied via a partition-shift SBUF DMA + DVE muls
    (cos/sin tables are host-provided, with the q tables pre-scaled by
    1/sqrt(dh) and the sin tables pre-signed for the rotate-half).
  - v computed in natural layout [tok, d].
  - scores per (head, 128-row tile of queries): one bf16 matmul per
    512-wide key tile (upper-triangle tiles skipped), bias added in
    natural layout: ACT computes log(1 + mem*gs + 1e-8), DVE folds
    prophetic*gs/2 + policy*gs + scores, gpsimd applies the causal
    fill on diagonal tiles via affine_select.
  - exp on ACT (bf16 out) with accum_out accumulating the softmax
    denominator for free.
  - P~ (bf16) transposed 128x128-blockwise via DMA XBAR transposes;
    AV matmul computes attn^T[d, tok] directly (lhsT = v tiles).
  - denominators transposed via a tiny PE transpose; reciprocal on DVE;
    normalization fused into the AV psum->sbuf copy.
  - out-projection over the core's 512 head-dims -> partial [S, D] f32.
"""

import sys

sys.path.insert(0, "/opt/trn_rl_repo")

import math

import ml_dtypes
import numpy as np

# problem shapes (hardcoded per contract)
B, S, D = 2, 1024, 2048
H, DH = 16, 128
GS = 0.1
ROPE_BASE = 10000.0
NCORES = 8
HPC = 4  # heads per core
DLOC = HPC * DH  # 512
TOK = S  # tokens per core (one batch each)
P = 128
KT = D // P  # 16 k-tiles over hidden dim
MASK_F32 = False  # stream governance masks as f32 (False: bf16, halves DMA)

_CACHE = {}


def legalize_sync_waits(nc, max_waits=1):
    """This walrus build only encodes 1 sem wait per instruction; move
    overflow waits onto same-engine NoOps placed immediately before."""
    import concourse.mybir as mybir

    n_split = 0
    for f in nc.m.functions:
        for bb in f.blocks:
            il = bb.instructions
            i = 0
            while i < len(il):
                inst = il[i]
                si = inst.sync_info
                if si is not None and len(si.on_wait) > max_waits:
                    waits = list(si.on_wait)
                    keep = waits[:max_waits]
                    extra = waits[max_waits:]
                    pos = i
                    j = 0
                    while extra:
                        chunk, extra = extra[:max_waits], extra[max_waits:]
                        nop = mybir.InstNoOp(
                            name=f"{inst.name}-swx{j}",
                            engine=inst.engine,
                            bass_nofuse=True,
                            sync_info=mybir.SyncInfo(on_wait=chunk, on_update=[]),
                        )
                        il.insert(pos, nop)
                        pos += 1
                        j += 1
                    inst.sync_info = mybir.SyncInfo(
                        on_wait=keep, on_update=list(si.on_update)
                    )
                    i = pos + 1
                    n_split += 1
                else:
                    i += 1
    return n_split


def build_nc():
    import concourse.bass as bass
    import concourse.mybir as mybir
    from concourse.masks import make_identity
    from concourse.tile import TileContext

    f32 = mybir.dt.float32
    bf16 = mybir.dt.bfloat16
    mask_dt = f32 if MASK_F32 else bf16
    Alu = mybir.AluOpType
    Act = mybir.ActivationFunctionType

    nc = bass.Bass()

    xt = nc.dram_tensor("xt", [D, TOK], bf16, kind="ExternalInput")
    wq = nc.dram_tensor("wq", [D, DLOC], bf16, kind="ExternalInput")
    wk = nc.dram_tensor("wk", [D, DLOC], bf16, kind="ExternalInput")
    wv = nc.dram_tensor("wv", [D, DLOC], bf16, kind="ExternalInput")
    wo = nc.dram_tensor("wo", [DLOC, D], bf16, kind="ExternalInput")
    pm = nc.dram_tensor("pm", [HPC, S, S], mask_dt, kind="ExternalInput")
    pol = nc.dram_tensor("pol", [HPC, S, S], mask_dt, kind="ExternalInput")
    mem = nc.dram_tensor("mem", [HPC, S, S], mask_dt, kind="ExternalInput")
    cosq = nc.dram_tensor("cosq", [P, TOK], f32, kind="ExternalInput")
    sinq = nc.dram_tensor("sinq", [P, TOK], f32, kind="ExternalInput")
    cosk = nc.dram_tensor("cosk", [P, TOK], f32, kind="ExternalInput")
    sink = nc.dram_tensor("sink", [P, TOK], f32, kind="ExternalInput")
    out = nc.dram_tensor("out", [TOK, D], f32, kind="ExternalOutput")

    xt_t = xt.rearrange("(kt p) t -> p kt t", p=P)
    wq_t = wq.rearrange("(kt p) c -> p kt c", p=P)
    wk_t = wk.rearrange("(kt p) c -> p kt c", p=P)
    wv_t = wv.rearrange("(kt p) c -> p kt c", p=P)
    wo_t = wo.rearrange("(kt p) c -> p kt c", p=P)

    with TileContext(nc) as tc:
        with (
            tc.tile_pool(name="persist", bufs=1) as persist,
            tc.tile_pool(name="ppsum", bufs=3, space="PSUM") as ppsum,
            tc.tile_pool(name="qk_psum", bufs=3, space="PSUM") as qk_psum,
            tc.tile_pool(name="av_psum", bufs=2, space="PSUM") as av_psum,
            tc.tile_pool(name="work", bufs=2) as work,
            tc.tile_pool(name="masks", bufs=2) as maskp,
            tc.tile_pool(name="pnatp", bufs=3) as pnatp,
            tc.tile_pool(name="ptp", bufs=1) as ptp,
            tc.tile_pool(name="lp", bufs=2) as lp,
        ):
            # ---- persistent tiles
            qT = persist.tile([P, HPC, TOK], bf16)  # [d, h, tok]
            kT = persist.tile([P, HPC, TOK], bf16)
            v_sb = persist.tile([P, TOK // P, DLOC], bf16)  # [tokp, tokt, hd]
            attnT = persist.tile([P, HPC, TOK], bf16)  # [d, h(=ktile), tok]
            lnbias = persist.tile([P, 1], f32)
            nc.vector.memset(lnbias[:], 1.0 + 1e-8)

            # ---- phase 1: projections (scoped pools free afterwards)
            with tc.tile_pool(name="proj", bufs=1) as proj, tc.tile_pool(
                name="wstream", bufs=2
            ) as wstream:
                xt_sb = proj.tile([P, KT, TOK], bf16)
                nc.sync.dma_start(xt_sb[:], xt_t)
                tabs = {}
                for name, tab in (
                    ("cosq", cosq),
                    ("sinq", sinq),
                    ("cosk", cosk),
                    ("sink", sink),
                ):
                    t = proj.tile([P, TOK], bf16, tag=name)
                    nc.gpsimd.dma_start(t[:], tab[:])  # casts f32 -> bf16
                    tabs[name] = t

                # v natural (resident weights)
                wv_sb = proj.tile([P, KT, DLOC], bf16, tag="wv")
                nc.sync.dma_start(wv_sb[:], wv_t)
                for mt in range(TOK // P):
                    ps = ppsum.tile([P, DLOC], f32, tag="pp")
                    for k in range(KT):
                        nc.tensor.matmul(
                            ps[:],
                            xt_sb[:, k, mt * P : (mt + 1) * P],
                            wv_sb[:, k, :],
                            start=(k == 0),
                            stop=(k == KT - 1),
                        )
                    nc.scalar.copy(v_sb[:, mt, :], ps[:])

                # q/k transposed + rope; weights streamed per head
                for which, wt in (("q", wq_t), ("k", wk_t)):
                    dest = qT if which == "q" else kT
                    cos_t = tabs["cosq" if which == "q" else "cosk"]
                    sin_t = tabs["sinq" if which == "q" else "sink"]
                    for m in range(HPC):  # head
                        w_cur = wstream.tile([P, KT, P], bf16, tag="wqk")
                        nc.sync.dma_start(w_cur[:], wt[:, :, m * P : (m + 1) * P])
                        for n in range(TOK // 512):
                            tsl = slice(n * 512, (n + 1) * 512)
                            ps = ppsum.tile([P, 512], f32, tag="pp")
                            for k in range(KT):
                                nc.tensor.matmul(
                                    ps[:],
                                    w_cur[:, k, :],
                                    xt_sb[:, k, tsl],
                                    start=(k == 0),
                                    stop=(k == KT - 1),
                                )
                            raw = work.tile([P, 512], f32, tag="rope_raw")
                            nc.scalar.copy(raw[:], ps[:])
                            swp = work.tile([P, 512], f32, tag="rope_swp")
                            nc.sync.dma_start(swp[0:64, :], raw[64:128, :])
                            nc.sync.dma_start(swp[64:128, :], raw[0:64, :])
                            t1 = work.tile([P, 512], f32, tag="rope_t1")
                            nc.vector.tensor_tensor(
                                t1[:], ps[:], cos_t[:, tsl], Alu.mult
                            )
                            t2 = work.tile([P, 512], f32, tag="rope_t2")
                            nc.vector.tensor_tensor(
                                t2[:], swp[:], sin_t[:, tsl], Alu.mult
                            )
                            nc.vector.tensor_tensor(
                                dest[:, m, tsl], t1[:], t2[:], Alu.add
                            )

            # ---- phase 2: attention per local head
            NEG = -1.0e30
            for h in range(HPC):
                pT = ptp.tile([P, TOK // P, TOK], bf16, tag="pt")
                l_all = lp.tile([P, TOK // P], f32, tag="l_all")
                for mig in range(TOK // P // 2):  # pairs of query row-tiles
                    n_sk = (mig // 2 + 1) * 512
                    rsl = slice(mig * 2 * P, (mig + 1) * 2 * P)
                    # merged row-pair mask loads (pol on the ACT hwdge queue)
                    mem_t = maskp.tile([P, 2, TOK], mask_dt, tag="memt")
                    nc.sync.dma_start(
                        mem_t[:, :, :n_sk],
                        mem[h, rsl, :n_sk].rearrange("(j p) c -> p j c", p=P),
                    )
                    pm_t = maskp.tile([P, 2, TOK], mask_dt, tag="pmt")
                    nc.sync.dma_start(
                        pm_t[:, :, :n_sk],
                        pm[h, rsl, :n_sk].rearrange("(j p) c -> p j c", p=P),
                    )
                    pol_t = maskp.tile([P, 2, TOK], mask_dt, tag="polt")
                    nc.scalar.dma_start(
                        pol_t[:, :, :n_sk],
                        pol[h, rsl, :n_sk].rearrange("(j p) c -> p j c", p=P),
                    )
                    for j in range(2):
                        mi = mig * 2 + j
                        pnat = pnatp.tile([P, TOK], bf16, tag="pnat")
                        for ni in range(n_sk // 512):
                            ksl = slice(ni * 512, (ni + 1) * 512)
                            ps = qk_psum.tile([P, 512], f32, tag="qk")
                            nc.tensor.matmul(
                                ps[:],
                                qT[:, h, mi * P : (mi + 1) * P],
                                kT[:, h, ksl],
                                start=True,
                                stop=True,
                            )
                            logm = work.tile([P, 512], f32, tag="logm")
                            nc.scalar.activation(
                                logm[:],
                                mem_t[:, j, ksl],
                                Act.Ln,
                                bias=lnbias[:],
                                scale=GS,
                            )
                            s1 = work.tile([P, 512], f32, tag="s1")
                            nc.vector.scalar_tensor_tensor(
                                s1[:],
                                pm_t[:, j, ksl],
                                GS * 0.5,
                                logm[:],
                                Alu.mult,
                                Alu.add,
                            )
                            s2 = work.tile([P, 512], f32, tag="s2")
                            nc.vector.scalar_tensor_tensor(
                                s2[:],
                                pol_t[:, j, ksl],
                                GS,
                                s1[:],
                                Alu.mult,
                                Alu.add,
                            )
                            s3 = work.tile([P, 512], f32, tag="s3")
                            nc.vector.tensor_tensor(s3[:], s2[:], ps[:], Alu.add)
                            if ni == mi // 4:  # diagonal tile: causal fill
                                nc.gpsimd.affine_select(
                                    out=s3[:],
                                    in_=s3[:],
                                    compare_op=Alu.is_ge,
                                    fill=NEG,
                                    base=mi * P - ni * 512,
                                    pattern=[[-1, 512]],
                                    channel_multiplier=1,
                                )
                            if ni == 0:
                                acc = l_all[:, mi : mi + 1]
                            else:
                                ltmp = lp.tile([P, 1], f32, tag="ltmp")
                                acc = ltmp[:]
                            nc.scalar.activation(
                                pnat[:, ksl],
                                s3[:],
                                Act.Exp,
                                accum_out=acc,
                            )
                            if ni == 1:
                                nc.vector.tensor_add(
                                    l_all[:, mi : mi + 1],
                                    l_all[:, mi : mi + 1],
                                    ltmp[:],
                                )
                        # normalize the row (per-partition 1/l), then one
                        # merged XBAR transpose into pT's column block
                        rec = lp.tile([P, 1], f32, tag="rec")
                        nc.vector.reciprocal(rec[:], l_all[:, mi : mi + 1])
                        nc.gpsimd.tensor_scalar_mul(
                            pnat[:, :n_sk], pnat[:, :n_sk], rec[:]
                        )
                        nc.scalar.dma_start_transpose(
                            pT[:, 0 : n_sk // P, mi * P : (mi + 1) * P],
                            pnat[:, :n_sk],
                        )
                # AV: attnT[d, tok] accumulating over key tiles
                for nj in range(2):
                    n_ki = 4 if nj == 0 else 8
                    ps = av_psum.tile([P, 512], f32, tag="av")
                    for ki in range(n_ki):
                        nc.tensor.matmul(
                            ps[:],
                            v_sb[:, ki, h * P : (h + 1) * P],
                            pT[:, ki, nj * 512 : (nj + 1) * 512],
                            start=(ki == 0),
                            stop=(ki == n_ki - 1),
                        )
                    nc.scalar.copy(attnT[:, h, nj * 512 : (nj + 1) * 512], ps[:])

            # ---- phase 3: out projection partial (wo streamed per n-chunk)
            with tc.tile_pool(name="outp", bufs=3) as outp, tc.tile_pool(
                name="wop", bufs=2
            ) as wop:
                for n in range(D // 512):
                    wo_sb = wop.tile([P, HPC, 512], bf16, tag="wo")
                    nc.sync.dma_start(
                        wo_sb[:], wo_t[:, :, n * 512 : (n + 1) * 512]
                    )
                    for mt in range(TOK // P):
                        ps = ppsum.tile([P, 512], f32, tag="pp")
                        for kt in range(HPC):
                            nc.tensor.matmul(
                                ps[:],
                                attnT[:, kt, mt * P : (mt + 1) * P],
                                wo_sb[:, kt, :],
                                start=(kt == 0),
                                stop=(kt == HPC - 1),
                            )
                        ot = outp.tile([P, 512], f32, tag="ot")
                        nc.scalar.copy(ot[:], ps[:])
                        nc.sync.dma_start(
                            out[mt * P : (mt + 1) * P, n * 512 : (n + 1) * 512],
                            ot[:],
                        )

    legalize_sync_waits(nc, max_waits=1)
    return nc


def _rope_tables():
    """cos/sin tables in transposed-projection layout [128 dims, TOK],
    with rotate-half sign folded into sin and 1/sqrt(dh) folded into the
    q tables."""
    inv_freq = 1.0 / (
        ROPE_BASE ** (np.arange(0, DH, 2, dtype=np.float32) / DH)
    )  # [64]
    t = np.arange(S, dtype=np.float32)
    freqs = np.outer(t, inv_freq)  # [S, 64]
    cos = np.cos(freqs)
    sin = np.sin(freqs)
    cos2 = np.empty((P, TOK), np.float32)
    sin2 = np.empty((P, TOK), np.float32)
    cos2[0:64] = cos.T
    cos2[64:128] = cos.T
    sin2[0:64] = -sin.T
    sin2[64:128] = sin.T
    scale = 1.0 / math.sqrt(DH)
    return cos2 * scale, sin2 * scale, cos2, sin2


def make_in_maps(x, prophetic_mask, policy_mask, memory_weights, Wq, Wk, Wv, Wo):
    from concurrent.futures import ThreadPoolExecutor

    bf16 = ml_dtypes.bfloat16
    mask_np_dt = np.float32 if MASK_F32 else bf16
    cosq, sinq, cosk, sink = _rope_tables()
    wq_b = np.ascontiguousarray(Wq).astype(bf16)
    wk_b = np.ascontiguousarray(Wk).astype(bf16)
    wv_b = np.ascontiguousarray(Wv).astype(bf16)
    wo_b = np.ascontiguousarray(Wo).astype(bf16)

    def shard(c):
        b = c // 4
        g = c % 4
        cols = slice(DLOC * g, DLOC * (g + 1))
        hsl = slice(HPC * g, HPC * (g + 1))
        return {
            "xt": np.ascontiguousarray(x[b].T).astype(bf16),
            "wq": np.ascontiguousarray(wq_b[:, cols]),
            "wk": np.ascontiguousarray(wk_b[:, cols]),
            "wv": np.ascontiguousarray(wv_b[:, cols]),
            "wo": np.ascontiguousarray(wo_b[cols, :]),
            "pm": prophetic_mask[b, hsl].astype(mask_np_dt),
            "pol": policy_mask[b, hsl].astype(mask_np_dt),
            "mem": memory_weights[b, hsl].astype(mask_np_dt),
            "cosq": cosq,
            "sinq": sinq,
            "cosk": cosk,
            "sink": sink,
        }

    with ThreadPoolExecutor(8) as ex:
        in_maps = list(ex.map(shard, range(NCORES)))
    return in_maps


def kernel(x, prophetic_mask, policy_mask, memory_weights, Wq, Wk, Wv, Wo, bo):
    from concourse.bass_utils import run_bass_kernel_spmd

    if "nc" not in _CACHE:
        _CACHE["nc"] = build_nc()
    nc = _CACHE["nc"]
    in_maps = make_in_maps(
        x, prophetic_mask, policy_mask, memory_weights, Wq, Wk, Wv, Wo
    )
    res = run_bass_kernel_spmd(nc, in_maps, list(range(NCORES)))
    out = np.zeros((B, S, D), np.float32)
    for c in range(NCORES):
        out[c // 4] += res.results[c]["out"]
    out += np.asarray(bo, np.float32)[None, None, :]
    return out


# revision 22
# speedup vs baseline: 29.7202x; 3.0053x over previous
"""EnhancedGovernanceAttention on 8 trn2 NeuronCores.

Sharding: tensor-parallel over heads with batch as secondary axis.
Core c handles batch b = c//4 and heads h in [4*(c%4), 4*(c%4)+4).
Each core computes a partial output [S, D] (its heads' contribution to
the out-projection); the host sums the partials and adds the bias.

Device kernel per core:
  - q,k projections computed transposed (qT[d, tok]) so QK^T needs no
    transpose; RoPE applied via a partition-shift SBUF DMA + DVE muls
    (cos/sin tables are host-provided, with the q tables pre-scaled by
    1/sqrt(dh) and the sin tables pre-signed for the rotate-half).
  - v computed in natural layout [tok, d].
  - scores per (head, 128-row tile of queries): one bf16 matmul per
    512-wide key tile (upper-triangle tiles skipped), bias added in
    natural layout: ACT computes log(1 + mem*gs + 1e-8), DVE folds
    prophetic*gs/2 + policy*gs + scores, gpsimd applies the causal
    fill on diagonal tiles via affine_select.
  - exp on ACT (bf16 out) with accum_out accumulating the softmax
    denominator for free.
  - P~ (bf16) transposed 128x128-blockwise via DMA XBAR transposes;
    AV matmul computes attn^T[d, tok] directly (lhsT = v tiles).
  - denominators transposed via a tiny PE transpose; reciprocal on DVE;
    normalization fused into the AV psum->sbuf copy.
  - out-projection over the core's 512 head-dims -> partial [S, D] f32.
"""

import sys

sys.path.insert(0, "/opt/trn_rl_repo")

import math

import ml_dtypes
import numpy as np

# problem shapes (hardcoded per contract)
B, S, D = 2, 1024, 2048
H, DH = 16, 128
GS = 0.1
ROPE_BASE = 10000.0
NCORES = 8
HPC = 4  # heads per core
DLOC = HPC * DH  # 512
TOK = S  # tokens per core (one batch each)
P = 128
KT = D // P  # 16 k-tiles over hidden dim
MASK_F32 = False  # stream governance masks as f32 (False: bf16, halves DMA)

_CACHE = {}


def legalize_sync_waits(nc, max_waits=1):
    """This walrus build only encodes 1 sem wait per instruction; move
    overflow waits onto same-engine NoOps placed immediately before."""
    import concourse.mybir as mybir

    n_split = 0
    for f in nc.m.functions:
        for bb in f.blocks:
            il = bb.instructions
            i = 0
            while i < len(il):
                inst = il[i]
                si = inst.sync_info
                if si is not None and len(si.on_wait) > max_waits:
                    waits = list(si.on_wait)
                    keep = waits[:max_waits]
                    extra = waits[max_waits:]
                    pos = i
                    j = 0
                    while extra:
                        chunk, extra = extra[:max_waits], extra[max_waits:]
                        nop = mybir.InstNoOp(
                            name=f"{inst.name}-swx{j}",
                            engine=inst.engine,
                            bass_nofuse=True,
                            sync_info=mybir.SyncInfo(on_wait=chunk, on_update=[]),
                        )
                        il.insert(pos, nop)
                        pos += 1
                        j += 1
                    inst.sync_info = mybir.SyncInfo(
                        on_wait=keep, on_update=list(si.on_update)
                    )
                    i = pos + 1
                    n_split += 1
                else:
                    i += 1
    return n_split


def build_nc(repeat=1):
    import concourse.bass as bass
    import concourse.mybir as mybir
    from concourse.tile import TileContext

    f32 = mybir.dt.float32
    bf16 = mybir.dt.bfloat16
    mask_dt = f32 if MASK_F32 else bf16
    Alu = mybir.AluOpType
    Act = mybir.ActivationFunctionType

    nc = bass.Bass()

    xt = nc.dram_tensor("xt", [D, TOK], bf16, kind="ExternalInput")
    wq = nc.dram_tensor("wq", [D, DLOC], bf16, kind="ExternalInput")
    wk = nc.dram_tensor("wk", [D, DLOC], bf16, kind="ExternalInput")
    wv = nc.dram_tensor("wv", [D, DLOC], bf16, kind="ExternalInput")
    wo = nc.dram_tensor("wo", [DLOC, D], bf16, kind="ExternalInput")
    pm = nc.dram_tensor("pm", [HPC, S, S], mask_dt, kind="ExternalInput")
    pol = nc.dram_tensor("pol", [HPC, S, S], mask_dt, kind="ExternalInput")
    mem = nc.dram_tensor("mem", [HPC, S, S], mask_dt, kind="ExternalInput")
    cosq = nc.dram_tensor("cosq", [P, TOK], f32, kind="ExternalInput")
    sinq = nc.dram_tensor("sinq", [P, TOK], f32, kind="ExternalInput")
    cosk = nc.dram_tensor("cosk", [P, TOK], f32, kind="ExternalInput")
    sink = nc.dram_tensor("sink", [P, TOK], f32, kind="ExternalInput")
    out = nc.dram_tensor("out", [TOK, D], f32, kind="ExternalOutput")

    xt_t = xt.rearrange("(kt p) t -> p kt t", p=P)
    wq_t = wq.rearrange("(kt p) c -> p kt c", p=P)
    wk_t = wk.rearrange("(kt p) c -> p kt c", p=P)
    wv_t = wv.rearrange("(kt p) c -> p kt c", p=P)
    wo_t = wo.rearrange("(kt p) c -> p kt c", p=P)

    with TileContext(nc) as tc:
        with (
            tc.tile_pool(name="persist", bufs=1) as persist,
            tc.tile_pool(name="ppsum", bufs=3, space="PSUM") as ppsum,
            tc.tile_pool(name="qk_psum", bufs=3, space="PSUM") as qk_psum,
            tc.tile_pool(name="av_psum", bufs=2, space="PSUM") as av_psum,
            tc.tile_pool(name="work", bufs=2) as work,
            tc.tile_pool(name="masks", bufs=2) as maskp,
            tc.tile_pool(name="pnatp", bufs=3) as pnatp,
            tc.tile_pool(name="ptp", bufs=1) as ptp,
            tc.tile_pool(name="lp", bufs=2) as lp,
        ):
            # ---- persistent tiles (repeat>1 is a timing-only mode that
            # runs the whole body multiple times in one NEFF)
            for _rep in range(repeat):
                body(nc, tc, locals())
    legalize_sync_waits(nc, max_waits=1)
    return nc


def body(nc, tc, env):
    import concourse.mybir as mybir

    f32 = mybir.dt.float32
    bf16 = mybir.dt.bfloat16
    mask_dt = f32 if MASK_F32 else bf16
    Alu = mybir.AluOpType
    Act = mybir.ActivationFunctionType
    (persist, ppsum, qk_psum, av_psum, work, maskp, pnatp, ptp, lp) = (
        env["persist"],
        env["ppsum"],
        env["qk_psum"],
        env["av_psum"],
        env["work"],
        env["maskp"],
        env["pnatp"],
        env["ptp"],
        env["lp"],
    )
    xt_t, wq_t, wk_t, wv_t, wo_t = (
        env["xt_t"],
        env["wq_t"],
        env["wk_t"],
        env["wv_t"],
        env["wo_t"],
    )
    cosq, sinq, cosk, sink = env["cosq"], env["sinq"], env["cosk"], env["sink"]
    pm, pol, mem, out = env["pm"], env["pol"], env["mem"], env["out"]
    if True:
        if True:
            # ---- persistent tiles
            qT = persist.tile([P, HPC, TOK], bf16)  # [d, h, tok]
            kT = persist.tile([P, HPC, TOK], bf16)
            v_sb = persist.tile([P, TOK // P, DLOC], bf16)  # [tokp, tokt, hd]
            attnT = persist.tile([P, HPC, TOK], bf16)  # [d, h(=ktile), tok]
            lnbias = persist.tile([P, 1], f32)
            nc.vector.memset(lnbias[:], 1.0 + 1e-8)

            # ---- phase 1: projections (scoped pools free afterwards)
            with tc.tile_pool(name="proj", bufs=1) as proj, tc.tile_pool(
                name="wstream", bufs=2
            ) as wstream:
                xt_sb = proj.tile([P, KT, TOK], bf16)
                nc.sync.dma_start(xt_sb[:], xt_t)
                tabs = {}
                for name, tab in (
                    ("cosq", cosq),
                    ("sinq", sinq),
                    ("cosk", cosk),
                    ("sink", sink),
                ):
                    t = proj.tile([P, TOK], bf16, tag=name)
                    nc.gpsimd.dma_start(t[:], tab[:])  # casts f32 -> bf16
                    tabs[name] = t

                # v natural (resident weights)
                wv_sb = proj.tile([P, KT, DLOC], bf16, tag="wv")
                nc.sync.dma_start(wv_sb[:], wv_t)
                for mt in range(TOK // P):
                    ps = ppsum.tile([P, DLOC], f32, tag="pp")
                    for k in range(KT):
                        nc.tensor.matmul(
                            ps[:],
                            xt_sb[:, k, mt * P : (mt + 1) * P],
                            wv_sb[:, k, :],
                            start=(k == 0),
                            stop=(k == KT - 1),
                        )
                    nc.scalar.copy(v_sb[:, mt, :], ps[:])

                # q/k transposed + rope; weights streamed per head
                for which, wt in (("q", wq_t), ("k", wk_t)):
                    dest = qT if which == "q" else kT
                    cos_t = tabs["cosq" if which == "q" else "cosk"]
                    sin_t = tabs["sinq" if which == "q" else "sink"]
                    for m in range(HPC):  # head
                        w_cur = wstream.tile([P, KT, P], bf16, tag="wqk")
                        nc.sync.dma_start(w_cur[:], wt[:, :, m * P : (m + 1) * P])
                        for n in range(TOK // 512):
                            tsl = slice(n * 512, (n + 1) * 512)
                            ps = ppsum.tile([P, 512], f32, tag="pp")
                            for k in range(KT):
                                nc.tensor.matmul(
                                    ps[:],
                                    w_cur[:, k, :],
                                    xt_sb[:, k, tsl],
                                    start=(k == 0),
                                    stop=(k == KT - 1),
                                )
                            raw = work.tile([P, 512], f32, tag="rope_raw")
                            nc.scalar.copy(raw[:], ps[:])
                            swp = work.tile([P, 512], f32, tag="rope_swp")
                            nc.sync.dma_start(swp[0:64, :], raw[64:128, :])
                            nc.sync.dma_start(swp[64:128, :], raw[0:64, :])
                            t1 = work.tile([P, 512], f32, tag="rope_t1")
                            nc.vector.tensor_tensor(
                                t1[:], ps[:], cos_t[:, tsl], Alu.mult
                            )
                            t2 = work.tile([P, 512], f32, tag="rope_t2")
                            nc.vector.tensor_tensor(
                                t2[:], swp[:], sin_t[:, tsl], Alu.mult
                            )
                            nc.vector.tensor_tensor(
                                dest[:, m, tsl], t1[:], t2[:], Alu.add
                            )

            # ---- phase 2: attention per local head
            NEG = -1.0e30
            negreg = nc.gpsimd.to_reg(NEG)
            for h in range(HPC):
                pT = ptp.tile([P, TOK // P, TOK], bf16, tag="pt")
                l_all = lp.tile([P, TOK // P], f32, tag="l_all")
                for mig in range(TOK // P // 2):  # pairs of query row-tiles
                    n_sk = (mig // 2 + 1) * 512
                    rsl = slice(mig * 2 * P, (mig + 1) * 2 * P)
                    # merged row-pair mask loads (pol on the ACT hwdge queue)
                    mem_t = maskp.tile([P, 2, TOK], mask_dt, tag="memt")
                    nc.sync.dma_start(
                        mem_t[:, :, :n_sk],
                        mem[h, rsl, :n_sk].rearrange("(j p) c -> p j c", p=P),
                    )
                    pm_t = maskp.tile([P, 2, TOK], mask_dt, tag="pmt")
                    nc.sync.dma_start(
                        pm_t[:, :, :n_sk],
                        pm[h, rsl, :n_sk].rearrange("(j p) c -> p j c", p=P),
                    )
                    pol_t = maskp.tile([P, 2, TOK], mask_dt, tag="polt")
                    nc.scalar.dma_start(
                        pol_t[:, :, :n_sk],
                        pol[h, rsl, :n_sk].rearrange("(j p) c -> p j c", p=P),
                    )
                    for j in range(2):
                        mi = mig * 2 + j
                        pnat = pnatp.tile([P, TOK], bf16, tag="pnat")
                        for ni in range(n_sk // 512):
                            ksl = slice(ni * 512, (ni + 1) * 512)
                            ps = qk_psum.tile([P, 512], f32, tag="qk")
                            nc.tensor.matmul(
                                ps[:],
                                qT[:, h, mi * P : (mi + 1) * P],
                                kT[:, h, ksl],
                                start=True,
                                stop=True,
                            )
                            logm = work.tile([P, 512], f32, tag="logm")
                            nc.scalar.activation(
                                logm[:],
                                mem_t[:, j, ksl],
                                Act.Ln,
                                bias=lnbias[:],
                                scale=GS,
                            )
                            s1 = work.tile([P, 512], f32, tag="s1")
                            nc.vector.scalar_tensor_tensor(
                                s1[:],
                                pm_t[:, j, ksl],
                                GS * 0.5,
                                logm[:],
                                Alu.mult,
                                Alu.add,
                            )
                            s2 = work.tile([P, 512], f32, tag="s2")
                            nc.vector.scalar_tensor_tensor(
                                s2[:],
                                pol_t[:, j, ksl],
                                GS,
                                s1[:],
                                Alu.mult,
                                Alu.add,
                            )
                            s3 = work.tile([P, 512], f32, tag="s3")
                            nc.vector.tensor_tensor(s3[:], s2[:], ps[:], Alu.add)
                            if ni == mi // 4:  # diagonal tile: causal fill
                                nc.gpsimd.affine_select(
                                    out=s3[:],
                                    in_=s3[:],
                                    compare_op=Alu.is_ge,
                                    fill=negreg,
                                    base=mi * P - ni * 512,
                                    pattern=[[-1, 512]],
                                    channel_multiplier=1,
                                )
                            if ni == 0:
                                acc = l_all[:, mi : mi + 1]
                            else:
                                ltmp = lp.tile([P, 1], f32, tag="ltmp")
                                acc = ltmp[:]
                            nc.scalar.activation(
                                pnat[:, ksl],
                                s3[:],
                                Act.Exp,
                                accum_out=acc,
                            )
                            if ni == 1:
                                nc.vector.tensor_add(
                                    l_all[:, mi : mi + 1],
                                    l_all[:, mi : mi + 1],
                                    ltmp[:],
                                )
                        # normalize the row (per-partition 1/l), then one
                        # merged XBAR transpose into pT's column block
                        rec = lp.tile([P, 1], f32, tag="rec")
                        nc.vector.reciprocal(rec[:], l_all[:, mi : mi + 1])
                        nc.gpsimd.tensor_scalar_mul(
                            pnat[:, :n_sk], pnat[:, :n_sk], rec[:]
                        )
                        nc.scalar.dma_start_transpose(
                            pT[:, 0 : n_sk // P, mi * P : (mi + 1) * P],
                            pnat[:, :n_sk],
                        )
                # AV: attnT[d, tok] accumulating over key tiles
                for nj in range(2):
                    n_ki = 4 if nj == 0 else 8
                    ps = av_psum.tile([P, 512], f32, tag="av")
                    for ki in range(n_ki):
                        nc.tensor.matmul(
                            ps[:],
                            v_sb[:, ki, h * P : (h + 1) * P],
                            pT[:, ki, nj * 512 : (nj + 1) * 512],
                            start=(ki == 0),
                            stop=(ki == n_ki - 1),
                        )
                    nc.scalar.copy(attnT[:, h, nj * 512 : (nj + 1) * 512], ps[:])

            # ---- phase 3: out projection partial (wo streamed per n-chunk)
            with tc.tile_pool(name="outp", bufs=3) as outp, tc.tile_pool(
                name="wop", bufs=2
            ) as wop:
                for n in range(D // 512):
                    wo_sb = wop.tile([P, HPC, 512], bf16, tag="wo")
                    nc.sync.dma_start(
                        wo_sb[:], wo_t[:, :, n * 512 : (n + 1) * 512]
                    )
                    for mt in range(TOK // P):
                        ps = ppsum.tile([P, 512], f32, tag="pp")
                        for kt in range(HPC):
                            nc.tensor.matmul(
                                ps[:],
                                attnT[:, kt, mt * P : (mt + 1) * P],
                                wo_sb[:, kt, :],
                                start=(kt == 0),
                                stop=(kt == HPC - 1),
                            )
                        ot = outp.tile([P, 512], f32, tag="ot")
                        nc.scalar.copy(ot[:], ps[:])
                        nc.sync.dma_start(
                            out[mt * P : (mt + 1) * P, n * 512 : (n + 1) * 512],
                            ot[:],
                        )


def _rope_tables():
    """cos/sin tables in transposed-projection layout [128 dims, TOK],
    with rotate-half sign folded into sin and 1/sqrt(dh) folded into the
    q tables."""
    inv_freq = 1.0 / (
        ROPE_BASE ** (np.arange(0, DH, 2, dtype=np.float32) / DH)
    )  # [64]
    t = np.arange(S, dtype=np.float32)
    freqs = np.outer(t, inv_freq)  # [S, 64]
    cos = np.cos(freqs)
    sin = np.sin(freqs)
    cos2 = np.empty((P, TOK), np.float32)
    sin2 = np.empty((P, TOK), np.float32)
    cos2[0:64] = cos.T
    cos2[64:128] = cos.T
    sin2[0:64] = -sin.T
    sin2[64:128] = sin.T
    scale = 1.0 / math.sqrt(DH)
    return cos2 * scale, sin2 * scale, cos2, sin2


def make_in_maps(x, prophetic_mask, policy_mask, memory_weights, Wq, Wk, Wv, Wo):
    from concurrent.futures import ThreadPoolExecutor

    bf16 = ml_dtypes.bfloat16
    mask_np_dt = np.float32 if MASK_F32 else bf16
    cosq, sinq, cosk, sink = _rope_tables()
    wq_b = np.ascontiguousarray(Wq).astype(bf16)
    wk_b = np.ascontiguousarray(Wk).astype(bf16)
    wv_b = np.ascontiguousarray(Wv).astype(bf16)
    wo_b = np.ascontiguousarray(Wo).astype(bf16)

    def shard(c):
        b = c // 4
        g = c % 4
        cols = slice(DLOC * g, DLOC * (g + 1))
        hsl = slice(HPC * g, HPC * (g + 1))
        return {
            "xt": np.ascontiguousarray(x[b].T).astype(bf16),
            "wq": np.ascontiguousarray(wq_b[:, cols]),
            "wk": np.ascontiguousarray(wk_b[:, cols]),
            "wv": np.ascontiguousarray(wv_b[:, cols]),
            "wo": np.ascontiguousarray(wo_b[cols, :]),
            "pm": prophetic_mask[b, hsl].astype(mask_np_dt),
            "pol": policy_mask[b, hsl].astype(mask_np_dt),
            "mem": memory_weights[b, hsl].astype(mask_np_dt),
            "cosq": cosq,
            "sinq": sinq,
            "cosk": cosk,
            "sink": sink,
        }

    with ThreadPoolExecutor(8) as ex:
        in_maps = list(ex.map(shard, range(NCORES)))
    return in_maps


def kernel(x, prophetic_mask, policy_mask, memory_weights, Wq, Wk, Wv, Wo, bo):
    from concourse.bass_utils import run_bass_kernel_spmd

    if "nc" not in _CACHE:
        _CACHE["nc"] = build_nc()
    nc = _CACHE["nc"]
    in_maps = make_in_maps(
        x, prophetic_mask, policy_mask, memory_weights, Wq, Wk, Wv, Wo
    )
    res = run_bass_kernel_spmd(nc, in_maps, list(range(NCORES)))
    out = np.zeros((B, S, D), np.float32)
    for c in range(NCORES):
        out[c // 4] += res.results[c]["out"]
    out += np.asarray(bo, np.float32)[None, None, :]
    return out
